# revision 23
# baseline (speedup 1.0000x reference)
"""Trainium2 Bass kernel: depthwise 3x3 stencil conv (SAME, zero-pad) + residual.

Math (per image, per channel):
    out[h,w] = sum_{dh,dw} k[dh,dw] * x[h+dh-1, w+dw-1]  +  x[h,w]

The fixed stencil k = [[1,0,-1],[0,1,0],[-1,0,1]] is rank-2:
    k = outer((1,0,-1),(1,0,-1)) + center(1)
so with t[h,w] = x[h-1,w] - x[h+1,w] (vertical pass):
    out[h,w] = 2*x[h,w] + t[h,w-1] - t[h,w+1]

Device computes out/2 = (beta/2)*x + t'[w-1] - t'[w+1] with t' = (V/2)^T @ x,
all bf16; host upconverts and multiplies by 2 (exact).

Layout: host packs each core's 4 images h-major: x_d[h, i*10752 + w*96 + ch]
([112, 43008] bf16).  One SBUF slab xs holds x, is updated in place
(x -> v -> out/2) and is the store source; ts holds t' with 96-col zero
halos per image block (10944 cols each).

Engine pipeline per image (1024-col drain groups, 4-deep PSUM pipeline
over 4 x [112,1024] fp32 tiles = all 8 banks):
    SP  ring: input loads (image0 split fine, others in quarters) +
              image-3 tail stores once the ring is empty
    PE      : t' = vt^T @ xs 512-col matmuls into PSUM, sem inc per group
    ACT     : PSUM -> ts bf16 drain copies (1x engine, ~10.5us/image);
              never waits on anything but PE, so it cannot stall the
              pipeline; fires one 2048-col tail store at the very end
    DVE     : op1 v = (beta/2) x + t'[w-1]; op2 out/2 = v - t'[w+1]
              (bf16 tensor_tensor, 2x_1P mode, in place over xs).
              This engine paces the kernel (~11.7us busy per image).
    GPSIMD  : bulk output stores via the SWDGE ring, throttled until
              most loads have landed (drain-count clock)

Measured DMA behavior that shaped this schedule: one HWDGE ring sustains
only ~283 GB/s/core on loads (HBM read latency exposed per descriptor);
a second concurrent queue of STORES brings the aggregate to ~430 GB/s
(the SDMA engines alternate packets 1:1 between queues), but two load
queues do NOT help, and that 1:1 round-robin means any store issued
while loads are pending steals exactly half the load bandwidth.  Since
the compute spine is fed by the loads, stores are held back (GPSIMD ring
+ THROTTLE_GRP) until the loads are nearly done, and the tail is spread
across three rings.
"""

import sys
import numpy as np

for _p in ("/opt/trn_rl_repo",):
    if _p not in sys.path:
        sys.path.insert(0, _p)

# ---------------- problem constants (hardcoded per contract) ----------------
N_CORES = 8
N, H, W, CH = 32, 112, 112, 96
IMGS = N // N_CORES                    # 4 images per core
C = W * CH                             # 10752 cols per image
COLS = IMGS * C                        # 43008 cols per core slab
PAD = CH                               # one w column = 96 cols
TSB = C + 2 * PAD                      # 10944 cols per ts image block
MM_N = 512                             # matmul chunk (one PSUM bank of fp32)
HALF = C // 2                          # 5376

# drain groups per image: uniform 1024-col groups (+ 512 remainder) so the
# PSUM pipeline can run 4 deep (4 x 1024 fp32 = all 8 PSUM banks), which
# removes the ACT bubble at image boundaries that a 2-deep 2048 ping-pong
# causes (PE could only run 2 groups ahead of the drains)
GRPN = [(k * 1024, 1024) for k in range(10)] + [(10240, 512)]
# image 0 starts with two 512-col groups so the first DVE op (which only
# needs [0,512) of t') launches one chain-step earlier
GRP0 = [(0, 512), (512, 512)] + [(k * 1024, 1024) for k in range(1, 10)] + [(10240, 512)]
GRPS = [GRP0, GRPN, GRPN, GRPN]
# flattened global group list: (image, col0, len)
GL = [(i, c0, ln) for i in range(IMGS) for (c0, ln) in GRPS[i]]


def _grp_idx(i, icol):
    """Global drain-group index (1-based count) covering t' interior column
    icol of image i (clamped into the image)."""
    icol = min(max(icol, 0), C - 1)
    base = sum(len(GRPS[j]) for j in range(i))
    for k, (c0, ln) in enumerate(GRPS[i]):
        if c0 <= icol < c0 + ln:
            return base + k + 1
    raise AssertionError

_CACHE = {}
LAST_RESULTS = None  # BassKernelResults of the most recent run (for test.py)


def _build_bass(beta):
    from concourse import bass, mybir

    bf16 = mybir.dt.bfloat16
    f32 = mybir.dt.float32
    nc = bass.Bass(debug=False)
    x_d = nc.declare_dram_parameter("x", [H, COLS], bf16, isOutput=False)
    v_d = nc.declare_dram_parameter("vmat", [H, H], bf16, isOutput=False)
    out_d = nc.declare_dram_parameter("out", [H, COLS], bf16, isOutput=True)

    vt = nc.alloc_sbuf_tensor("vt", [H, H], bf16)
    xs = nc.alloc_sbuf_tensor("xs", [H, COLS], bf16)
    ts = nc.alloc_sbuf_tensor("ts", [H, IMGS * TSB], bf16)
    scr = nc.alloc_sbuf_tensor("scr", [H, 128], bf16)
    ps = [nc.alloc_psum_tensor(f"ps{b}", [H, 1024], f32) for b in range(4)]

    # input loads (col ranges of x_d/xs); image 0 split finer so PE starts
    # early; later images in quarters so PE/ACT/DVE see smooth arrivals
    LOADS = [(0, 512), (512, 2048), (2048, 4096), (4096, 5376),
             (5376, 8064), (8064, 10752)]
    for i in range(1, IMGS):
        for q in range(4):
            LOADS.append((i * C + q * 2688, i * C + (q + 1) * 2688))

    # DVE batches: (wait_group_count, [ops], inc) with op = (which, i, lo, hi)
    # op1 piece [lo,hi) reads t' interior [lo-96, hi-96); op2 reads [lo+96, hi+96)
    DVE_BATCHES = []

    def _b(i, ops, inc):
        need = 0
        for which, lo, hi in ops:
            icol = hi - 1 - PAD if which == 1 else hi - 1 + PAD
            need = max(need, _grp_idx(i, icol))
        DVE_BATCHES.append((need, [(w, i, lo, hi) for (w, lo, hi) in ops], inc))

    _b(0, [(1, 0, 512)], False)
    _b(0, [(1, 512, 1024), (2, 0, 512)], False)
    _b(0, [(1, 1024, 2048), (2, 512, 1024)], False)
    _b(0, [(1, 2048, 4096), (2, 1024, 2048)], True)       # inc 1
    _b(0, [(1, 4096, 6144), (2, 2048, 4096)], True)       # inc 2
    _b(0, [(1, 6144, 8192), (2, 4096, 6144)], True)       # inc 3
    _b(0, [(1, 8192, 10240), (2, 6144, 8192)], True)      # inc 4
    _b(0, [(1, 10240, C), (2, 8192, 10240), (2, 10240, C)], True)  # inc 5
    for i in (1, 2):
        _b(i, [(1, 0, HALF), (2, 0, HALF)], True)
        _b(i, [(1, HALF, C), (2, HALF, C)], True)
    _b(3, [(1, 0, HALF), (2, 0, HALF)], True)             # inc 10
    _b(3, [(1, HALF, C), (2, HALF, 7552)], True)          # inc 11
    _b(3, [(2, 7552, 9600)], True)                        # inc 12
    _b(3, [(2, 9600, C)], True)                           # inc 13

    # stores: (image, col0, len, dve_count_required).
    # Loads sustain only ~283 GB/s on their ring and every concurrent store
    # packet steals load bandwidth 1:1 (SDMA round-robin), so stores are
    # issued from the otherwise-idle GPSIMD (SWDGE) ring -- its sem waits
    # block nobody -- and throttled until most loads are in (s_act clock).
    # Image 3's tail pieces ride the by-then-empty SP ring (HWDGE latency
    # beats SWDGE's ~4.5us/DMA descriptor emission, which matters at the
    # tail).
    GP_STORES = [
        (0, 0, 6144, 3), (0, 6144, 4608, 5),
        (1, 0, HALF, 6), (1, HALF, HALF, 7),
        (2, 0, HALF, 8), (2, HALF, HALF, 9),
    ]
    SP_STORES = [(3, 0, HALF, 10), (3, HALF, 2176, 11), (3, 9600, 1152, 13)]
    ACT_TAIL_STORE = (3, 7552, 2048, 12)  # ACT ring is idle by then
    THROTTLE_GRP = 25  # gp stores wait for this many drain groups first
    N_ST = len(GP_STORES) + len(SP_STORES) + 1

    from contextlib import ExitStack

    with (
        nc.Block(no_gpsimd_drain=True) as block,
        nc.semaphore("s_vt") as s_vt,
        nc.semaphore("s_pe") as s_pe,
        nc.semaphore("s_act") as s_act,
        nc.semaphore("s_dve") as s_dve,
        nc.semaphore("s_st") as s_st,
        ExitStack() as _sems,
    ):
        s_ld = [
            _sems.enter_context(nc.semaphore(f"s_ld{u}")) for u in range(len(LOADS))
        ]

        @block.sync
        def _(sp: bass.BassEngine):
            a0, b0 = LOADS[0]
            sp.dma_start(out=xs[:, a0:b0], in_=x_d[:, a0:b0]).then_inc(s_ld[0], 16)
            sp.dma_start(out=vt[:, :], in_=v_d[:, :]).then_inc(s_vt, 16)
            for u, (a, b) in enumerate(LOADS):
                if u == 0:
                    continue
                sp.dma_start(out=xs[:, a:b], in_=x_d[:, a:b]).then_inc(s_ld[u], 16)
            # image-3 stores ride the (by now idle) SP ring, overlapping the
            # ACT ring's earlier stores at the tail
            for i, c0, ln, req in SP_STORES:
                sp.wait_ge(s_dve, req)
                sp.dma_start(
                    out=out_d[:, i * C + c0 : i * C + c0 + ln],
                    in_=xs[:, i * C + c0 : i * C + c0 + ln],
                ).then_inc(s_st, 16)
            sp.wait_ge(s_st, 16 * N_ST)

        @block.tensor
        def _(pe: bass.BassEngine):
            pe.wait_ge(s_vt, 16)
            waited = [False] * len(LOADS)

            def need_cols(hi):
                for u, (a, b) in enumerate(LOADS):
                    if a < hi and not waited[u]:
                        pe.wait_ge(s_ld[u], 16)
                        waited[u] = True

            for gg, (i, g0, gsz) in enumerate(GL):
                base = i * C + g0
                need_cols(base + gsz)
                if gg >= 4:
                    pe.wait_ge(s_act, gg - 3)
                nch = gsz // MM_N if gsz >= MM_N else 1
                csz = gsz // nch
                for c in range(nch):
                    mm = pe.matmul(
                        out=ps[gg % 4][0:H, c * csz : (c + 1) * csz],
                        lhsT=vt[:, :],
                        rhs=xs[:, base + c * csz : base + (c + 1) * csz],
                        start=True,
                        stop=True,
                    )
                    if c == nch - 1:
                        mm.then_inc(s_pe, 1)

        @block.scalar
        def _(act: bass.BassEngine):
            # warm the activation table during the preamble; drains only --
            # no DMA waits on this engine, it must never stall
            act.mul(scr[:, 64:66], ps[0][0:H, 0:2], 0.5)
            for gg, (i, g0, gsz) in enumerate(GL):
                act.wait_ge(s_pe, gg + 1)
                act.copy(
                    ts[:, i * TSB + PAD + g0 : i * TSB + PAD + g0 + gsz],
                    ps[gg % 4][0:H, 0:gsz],
                ).then_inc(s_act, 1)
            i, c0, ln, req = ACT_TAIL_STORE
            act.wait_ge(s_dve, req)
            act.dma_start(
                out=out_d[:, i * C + c0 : i * C + c0 + ln],
                in_=xs[:, i * C + c0 : i * C + c0 + ln],
            ).then_inc(s_st, 16)

        @block.gpsimd
        def _(gp: bass.BassEngine):
            gp.wait_ge(s_act, THROTTLE_GRP)
            for i, c0, ln, req in GP_STORES:
                gp.wait_ge(s_dve, req)
                gp.dma_start(
                    out=out_d[:, i * C + c0 : i * C + c0 + ln],
                    in_=xs[:, i * C + c0 : i * C + c0 + ln],
                ).then_inc(s_st, 16)

        @block.vector
        def _(dve: bass.BassEngine):
            # zero the per-image halo slivers of ts once (DVE-local ordering)
            for i in range(IMGS):
                dve.memset(ts[:, i * TSB : i * TSB + PAD], 0.0)
                dve.memset(ts[:, i * TSB + PAD + C : (i + 1) * TSB], 0.0)

            def op1(i, lo, hi):
                # v = (beta/2)*x + t'[w-1]  over image-i interior [lo, hi)
                if beta == 2.0:
                    dve.tensor_tensor(
                        out=xs[:, i * C + lo : i * C + hi],
                        in0=xs[:, i * C + lo : i * C + hi],
                        in1=ts[:, i * TSB + lo : i * TSB + hi],
                        op=mybir.AluOpType.add,
                    )
                else:
                    dve.scalar_tensor_tensor(
                        out=xs[:, i * C + lo : i * C + hi],
                        in0=xs[:, i * C + lo : i * C + hi],
                        scalar=float(beta) / 2.0,
                        in1=ts[:, i * TSB + lo : i * TSB + hi],
                        op0=mybir.AluOpType.mult,
                        op1=mybir.AluOpType.add,
                    )

            def op2(i, lo, hi):
                dve.tensor_tensor(
                    out=xs[:, i * C + lo : i * C + hi],
                    in0=xs[:, i * C + lo : i * C + hi],
                    in1=ts[:, i * TSB + 2 * PAD + lo : i * TSB + 2 * PAD + hi],
                    op=mybir.AluOpType.subtract,
                )

            for need, ops, inc in DVE_BATCHES:
                dve.wait_ge(s_act, need)
                for which, i, lo, hi in ops:
                    (op1 if which == 1 else op2)(i, lo, hi)
                if inc:
                    dve.drain().then_inc(s_dve, 1)

    return nc


def _stencil_params(kern):
    """Validate the depthwise kernel and extract (vertical profile a, beta)."""
    k = np.asarray(kern, dtype=np.float32)
    if k.ndim != 4 or k.shape != (3, 3, 1, CH):
        return None
    if not np.all(k == k[:, :, :, :1]):
        return None
    k2 = k[:, :, 0, 0]
    if not (np.all(k2[:, 2] == -k2[:, 0]) and k2[0, 1] == 0 and k2[2, 1] == 0):
        return None
    return k2[:, 0].copy(), float(k2[1, 1]) + 1.0


def _numpy_fallback(x, kern):
    """Straightforward shifted-add implementation (safety net only)."""
    k = np.asarray(kern, dtype=np.float32)[:, :, 0, :]  # (3,3,CH)
    xp = np.pad(x, ((0, 0), (1, 1), (1, 1), (0, 0)))
    out = x.astype(np.float32).copy()
    for dh in range(3):
        for dw in range(3):
            out += k[dh, dw] * xp[:, dh : dh + H, dw : dw + W, :]
    return out


def _ensure_ntff_hook():
    """The agent image's antenv lacks axon_hooks; synthesize it so
    run_bass_kernel_spmd(trace=True) can reach the NTFF profiler."""
    import types

    if "antenv.axon_hooks" in sys.modules:
        return
    import antenv

    mod = types.ModuleType("antenv.axon_hooks")
    state = {}
    mod.set_axon_ntff_profile_hook = lambda h: state.__setitem__("h", h)
    mod.get_axon_ntff_profile_hook = lambda: state.get("h")
    sys.modules["antenv.axon_hooks"] = mod
    antenv.axon_hooks = mod
    try:
        if "/root/.axon_site" not in sys.path:
            sys.path.insert(0, "/root/.axon_site")
        from trn_agent_boot.trn_boot import _ntff_profile_via_ctypes

        hook = _ntff_profile_via_ctypes("/opt/axon/libaxon_pjrt.so")
        if hook is not None:
            mod.set_axon_ntff_profile_hook(hook)
    except Exception:
        pass


def _run_on_hw(x, a, beta, trace=False):
    global LAST_RESULTS
    if trace:
        _ensure_ntff_hook()
    import ml_dtypes
    from concourse.bass_utils import run_bass_kernel_spmd

    bf16 = ml_dtypes.bfloat16

    # vertical banded matrix scaled by 0.5: V[i, j] = coeff of x-row i in
    # t'-row j (t' = t/2)
    V = np.zeros((H, H), dtype=np.float32)
    idx = np.arange(H)
    V[idx[:-1] + 1, idx[:-1]] += a[2]
    V[idx, idx] += a[1]
    V[idx[1:] - 1, idx[1:]] += a[0]
    Vb = (0.5 * V).astype(bf16)

    key = (a.tobytes(), float(beta))
    if key not in _CACHE:
        _CACHE[key] = _build_bass(float(beta))
    nc = _CACHE[key]

    # h-major pack: core c gets [112, 4*10752] with images side by side
    xb = np.ascontiguousarray(
        x.reshape(N_CORES, IMGS, H, C).astype(bf16).transpose(0, 2, 1, 3)
    ).reshape(N_CORES, H, COLS)
    in_maps = [{"x": xb[c], "vmat": Vb} for c in range(N_CORES)]
    res = run_bass_kernel_spmd(nc, in_maps, list(range(N_CORES)), trace=trace)
    LAST_RESULTS = res
    # device returned out/2 in bf16 (h-major); unpack + x2 (exact in fp32)
    o = np.stack(
        [np.asarray(res.results[c]["out"]) for c in range(N_CORES)]
    )  # [8, 112, 43008] bf16
    out = (
        o.reshape(N_CORES, H, IMGS, C)
        .transpose(0, 2, 1, 3)
        .astype(np.float32)
        .reshape(N, H, W, CH)
    )
    out *= 2.0
    return out


def kernel(x, kernel=None, _trace=False, **_unused):
    x = np.ascontiguousarray(np.asarray(x, dtype=np.float32))
    assert x.shape == (N, H, W, CH), f"unexpected x shape {x.shape}"
    if kernel is None:
        base = np.array(
            [[1.0, 0.0, -1.0], [0.0, 1.0, 0.0], [-1.0, 0.0, 1.0]], dtype=np.float32
        )
        kernel = np.tile(base[:, :, None, None], (1, 1, 1, CH))
    params = _stencil_params(kernel)
    if params is None:
        return _numpy_fallback(x, kernel)
    a, beta = params
    return _run_on_hw(x, a, beta, trace=_trace)


if __name__ == "__main__":
    xs = np.random.randn(N, H, W, CH).astype(np.float32)
    out = kernel(xs)
    print(out.shape, out.dtype)


# revision 24
# speedup vs baseline: 1.0434x; 1.0434x over previous
"""Trainium2 Bass kernel: depthwise 3x3 stencil conv (SAME, zero-pad) + residual.

Math (per image, per channel):
    out[h,w] = sum_{dh,dw} k[dh,dw] * x[h+dh-1, w+dw-1]  +  x[h,w]

The fixed stencil k = [[1,0,-1],[0,1,0],[-1,0,1]] is rank-2:
    k = outer((1,0,-1),(1,0,-1)) + center(1)
so with t[h,w] = x[h-1,w] - x[h+1,w] (vertical pass):
    out[h,w] = 2*x[h,w] + t[h,w-1] - t[h,w+1]

Device computes out/2 = (beta/2)*x + t'[w-1] - t'[w+1] with t' = (V/2)^T @ x,
all bf16; host upconverts and multiplies by 2 (exact).

Layout: host packs each core's 4 images h-major: x_d[h, i*10752 + w*96 + ch]
([112, 43008] bf16).  One SBUF slab xs holds x, is updated in place
(x -> v -> out/2) and is the store source; ts holds t' with 96-col zero
halos per image block (10944 cols each).

Engine pipeline per image (1024-col drain groups, 4-deep PSUM pipeline
over 4 x [112,1024] fp32 tiles = all 8 banks):
    SP  ring: input loads (image0 split fine, others in thirds) + the
              image-3 tail stores once the ring is empty
    PE      : t' = vt^T @ xs 512-col matmuls into PSUM, sem inc per group
    ACT     : PSUM -> ts bf16 drain copies (1x engine, ~10.5us/image);
              never waits on anything but PE, so it cannot stall the
              pipeline; fires one 2688-col tail store at the very end
    DVE     : op1 v = (beta/2) x + t'[w-1]; op2 out/2 = v - t'[w+1]
              (bf16 tensor_tensor, 2x_1P mode, in place over xs).
              This engine paces the kernel (~11.7us busy per image).
    GPSIMD  : bulk output stores via the SWDGE ring, throttled until
              most loads have landed (drain-count clock)

Measured DMA behavior that shaped this schedule: one HWDGE ring sustains
only ~283 GB/s/core on loads (HBM read latency exposed per descriptor);
a second concurrent queue of STORES brings the aggregate to ~430 GB/s
(the SDMA engines alternate packets 1:1 between queues), but two load
queues do NOT help, and that 1:1 round-robin means any store issued
while loads are pending steals exactly half the load bandwidth.  Since
the compute spine is fed by the loads, stores are held back (GPSIMD ring
+ THROTTLE_GRP) until the loads are nearly done, and the tail is spread
across three rings.
"""

import sys
import numpy as np

for _p in ("/opt/trn_rl_repo",):
    if _p not in sys.path:
        sys.path.insert(0, _p)

# ---------------- problem constants (hardcoded per contract) ----------------
N_CORES = 8
N, H, W, CH = 32, 112, 112, 96
IMGS = N // N_CORES                    # 4 images per core
C = W * CH                             # 10752 cols per image
COLS = IMGS * C                        # 43008 cols per core slab
PAD = CH                               # one w column = 96 cols
TSB = C + 2 * PAD                      # 10944 cols per ts image block
MM_N = 512                             # matmul chunk (one PSUM bank of fp32)
HALF = C // 2                          # 5376

# drain groups per image: uniform 1024-col groups (+ 512 remainder) so the
# PSUM pipeline can run 4 deep (4 x 1024 fp32 = all 8 PSUM banks), which
# removes the ACT bubble at image boundaries that a 2-deep 2048 ping-pong
# causes (PE could only run 2 groups ahead of the drains)
GRPN = [(k * 1024, 1024) for k in range(10)] + [(10240, 512)]
GRPS = [GRPN, GRPN, GRPN, GRPN]
# flattened global group list: (image, col0, len)
GL = [(i, c0, ln) for i in range(IMGS) for (c0, ln) in GRPS[i]]


def _grp_idx(i, icol):
    """Global drain-group index (1-based count) covering t' interior column
    icol of image i (clamped into the image)."""
    icol = min(max(icol, 0), C - 1)
    base = sum(len(GRPS[j]) for j in range(i))
    for k, (c0, ln) in enumerate(GRPS[i]):
        if c0 <= icol < c0 + ln:
            return base + k + 1
    raise AssertionError

_CACHE = {}
LAST_RESULTS = None  # BassKernelResults of the most recent run (for test.py)


def _build_bass(beta):
    from concourse import bass, mybir

    bf16 = mybir.dt.bfloat16
    f32 = mybir.dt.float32
    nc = bass.Bass(debug=False)
    x_d = nc.declare_dram_parameter("x", [H, COLS], bf16, isOutput=False)
    v_d = nc.declare_dram_parameter("vmat", [H, H], bf16, isOutput=False)
    out_d = nc.declare_dram_parameter("out", [H, COLS], bf16, isOutput=True)

    vt = nc.alloc_sbuf_tensor("vt", [H, H], bf16)
    xs = nc.alloc_sbuf_tensor("xs", [H, COLS], bf16)
    ts = nc.alloc_sbuf_tensor("ts", [H, IMGS * TSB], bf16)
    scr = nc.alloc_sbuf_tensor("scr", [H, 128], bf16)
    ps = [nc.alloc_psum_tensor(f"ps{b}", [H, 1024], f32) for b in range(4)]

    # input loads (col ranges of x_d/xs); image 0 split finer so PE starts
    # early; later images in thirds so PE/ACT/DVE see smooth arrivals
    LOADS = [(0, 1024), (1024, 2048), (2048, 4096), (4096, 5376),
             (5376, 8064), (8064, 10752)]
    for i in range(1, IMGS):
        for q in range(4):
            LOADS.append((i * C + q * 2688, i * C + (q + 1) * 2688))

    # DVE batches: (wait_group_count, [ops], inc) with op = (which, i, lo, hi)
    # op1 piece [lo,hi) reads t' interior [lo-96, hi-96); op2 reads [lo+96, hi+96)
    DVE_BATCHES = []

    def _b(i, ops, inc):
        need = 0
        for which, lo, hi in ops:
            icol = hi - 1 - PAD if which == 1 else hi - 1 + PAD
            need = max(need, _grp_idx(i, icol))
        DVE_BATCHES.append((need, [(w, i, lo, hi) for (w, lo, hi) in ops], inc))

    _b(0, [(1, 0, 1024)], False)
    _b(0, [(1, 1024, 2048), (2, 0, 1024)], True)          # inc 1
    _b(0, [(1, 2048, 4096), (2, 1024, 2048)], True)       # inc 2 -> store [0,2048)
    _b(0, [(1, 4096, 6144), (2, 2048, 4096)], True)       # inc 3
    _b(0, [(1, 6144, 8192), (2, 4096, 6144)], True)       # inc 4
    _b(0, [(1, 8192, 10240), (2, 6144, 8192)], True)      # inc 5
    _b(0, [(1, 10240, C), (2, 8192, 10240), (2, 10240, C)], True)  # inc 6
    for i in (1, 2):
        _b(i, [(1, 0, HALF), (2, 0, HALF)], True)
        _b(i, [(1, HALF, C), (2, HALF, C)], True)
    _b(3, [(1, 0, HALF), (2, 0, HALF)], True)             # inc 11
    _b(3, [(1, HALF, C), (2, HALF, 7552)], True)          # inc 12
    _b(3, [(2, 7552, 9600)], True)                        # inc 13
    _b(3, [(2, 9600, C)], True)                           # inc 14

    # stores: (image, col0, len, dve_count_required).
    # Loads sustain only ~283 GB/s on their ring and every concurrent store
    # packet steals load bandwidth 1:1 (SDMA round-robin), so stores are
    # issued from the otherwise-idle GPSIMD (SWDGE) ring -- its sem waits
    # block nobody -- and throttled until most loads are in (s_act clock).
    # Image 3's tail pieces ride the by-then-empty SP ring (HWDGE latency
    # beats SWDGE's ~4.5us/DMA descriptor emission, which matters at the
    # tail).
    GP_STORES = [
        (0, 0, 6144, 4), (0, 6144, 4608, 6),
        (1, 0, HALF, 7), (1, HALF, HALF, 8),
        (2, 0, HALF, 9), (2, HALF, HALF, 10),
    ]
    SP_STORES = [(3, 0, HALF, 11), (3, HALF, 2176, 12), (3, 9600, 1152, 14)]
    ACT_TAIL_STORE = (3, 7552, 2048, 13)  # ACT ring is idle by then
    THROTTLE_GRP = 25  # gp stores wait for this many drain groups first
    N_ST = len(GP_STORES) + len(SP_STORES) + 1

    from contextlib import ExitStack

    with (
        nc.Block(no_gpsimd_drain=True) as block,
        nc.semaphore("s_vt") as s_vt,
        nc.semaphore("s_pe") as s_pe,
        nc.semaphore("s_act") as s_act,
        nc.semaphore("s_dve") as s_dve,
        nc.semaphore("s_st") as s_st,
        ExitStack() as _sems,
    ):
        s_ld = [
            _sems.enter_context(nc.semaphore(f"s_ld{u}")) for u in range(len(LOADS))
        ]

        @block.sync
        def _(sp: bass.BassEngine):
            a0, b0 = LOADS[0]
            sp.dma_start(out=xs[:, a0:b0], in_=x_d[:, a0:b0]).then_inc(s_ld[0], 16)
            sp.dma_start(out=vt[:, :], in_=v_d[:, :]).then_inc(s_vt, 16)
            for u, (a, b) in enumerate(LOADS):
                if u == 0:
                    continue
                sp.dma_start(out=xs[:, a:b], in_=x_d[:, a:b]).then_inc(s_ld[u], 16)
            # image-3 stores ride the (by now idle) SP ring, overlapping the
            # ACT ring's earlier stores at the tail
            for i, c0, ln, req in SP_STORES:
                sp.wait_ge(s_dve, req)
                sp.dma_start(
                    out=out_d[:, i * C + c0 : i * C + c0 + ln],
                    in_=xs[:, i * C + c0 : i * C + c0 + ln],
                ).then_inc(s_st, 16)
            sp.wait_ge(s_st, 16 * N_ST)

        @block.tensor
        def _(pe: bass.BassEngine):
            pe.wait_ge(s_vt, 16)
            waited = [False] * len(LOADS)

            def need_cols(hi):
                for u, (a, b) in enumerate(LOADS):
                    if a < hi and not waited[u]:
                        pe.wait_ge(s_ld[u], 16)
                        waited[u] = True

            for gg, (i, g0, gsz) in enumerate(GL):
                base = i * C + g0
                need_cols(base + gsz)
                if gg >= 4:
                    pe.wait_ge(s_act, gg - 3)
                nch = gsz // MM_N if gsz >= MM_N else 1
                csz = gsz // nch
                for c in range(nch):
                    mm = pe.matmul(
                        out=ps[gg % 4][0:H, c * csz : (c + 1) * csz],
                        lhsT=vt[:, :],
                        rhs=xs[:, base + c * csz : base + (c + 1) * csz],
                        start=True,
                        stop=True,
                    )
                    if c == nch - 1:
                        mm.then_inc(s_pe, 1)

        @block.scalar
        def _(act: bass.BassEngine):
            # warm the activation table during the preamble; drains only --
            # no DMA waits on this engine, it must never stall
            act.mul(scr[:, 64:66], ps[0][0:H, 0:2], 0.5)
            for gg, (i, g0, gsz) in enumerate(GL):
                act.wait_ge(s_pe, gg + 1)
                act.copy(
                    ts[:, i * TSB + PAD + g0 : i * TSB + PAD + g0 + gsz],
                    ps[gg % 4][0:H, 0:gsz],
                ).then_inc(s_act, 1)
            i, c0, ln, req = ACT_TAIL_STORE
            act.wait_ge(s_dve, req)
            act.dma_start(
                out=out_d[:, i * C + c0 : i * C + c0 + ln],
                in_=xs[:, i * C + c0 : i * C + c0 + ln],
            ).then_inc(s_st, 16)

        @block.gpsimd
        def _(gp: bass.BassEngine):
            gp.wait_ge(s_act, THROTTLE_GRP)
            for i, c0, ln, req in GP_STORES:
                gp.wait_ge(s_dve, req)
                gp.dma_start(
                    out=out_d[:, i * C + c0 : i * C + c0 + ln],
                    in_=xs[:, i * C + c0 : i * C + c0 + ln],
                ).then_inc(s_st, 16)

        @block.vector
        def _(dve: bass.BassEngine):
            # zero the per-image halo slivers of ts once (DVE-local ordering)
            for i in range(IMGS):
                dve.memset(ts[:, i * TSB : i * TSB + PAD], 0.0)
                dve.memset(ts[:, i * TSB + PAD + C : (i + 1) * TSB], 0.0)

            def op1(i, lo, hi):
                # v = (beta/2)*x + t'[w-1]  over image-i interior [lo, hi)
                if beta == 2.0:
                    dve.tensor_tensor(
                        out=xs[:, i * C + lo : i * C + hi],
                        in0=xs[:, i * C + lo : i * C + hi],
                        in1=ts[:, i * TSB + lo : i * TSB + hi],
                        op=mybir.AluOpType.add,
                    )
                else:
                    dve.scalar_tensor_tensor(
                        out=xs[:, i * C + lo : i * C + hi],
                        in0=xs[:, i * C + lo : i * C + hi],
                        scalar=float(beta) / 2.0,
                        in1=ts[:, i * TSB + lo : i * TSB + hi],
                        op0=mybir.AluOpType.mult,
                        op1=mybir.AluOpType.add,
                    )

            def op2(i, lo, hi):
                dve.tensor_tensor(
                    out=xs[:, i * C + lo : i * C + hi],
                    in0=xs[:, i * C + lo : i * C + hi],
                    in1=ts[:, i * TSB + 2 * PAD + lo : i * TSB + 2 * PAD + hi],
                    op=mybir.AluOpType.subtract,
                )

            for need, ops, inc in DVE_BATCHES:
                dve.wait_ge(s_act, need)
                for which, i, lo, hi in ops:
                    (op1 if which == 1 else op2)(i, lo, hi)
                if inc:
                    dve.drain().then_inc(s_dve, 1)

    return nc


def _stencil_params(kern):
    """Validate the depthwise kernel and extract (vertical profile a, beta)."""
    k = np.asarray(kern, dtype=np.float32)
    if k.ndim != 4 or k.shape != (3, 3, 1, CH):
        return None
    if not np.all(k == k[:, :, :, :1]):
        return None
    k2 = k[:, :, 0, 0]
    if not (np.all(k2[:, 2] == -k2[:, 0]) and k2[0, 1] == 0 and k2[2, 1] == 0):
        return None
    return k2[:, 0].copy(), float(k2[1, 1]) + 1.0


def _numpy_fallback(x, kern):
    """Straightforward shifted-add implementation (safety net only)."""
    k = np.asarray(kern, dtype=np.float32)[:, :, 0, :]  # (3,3,CH)
    xp = np.pad(x, ((0, 0), (1, 1), (1, 1), (0, 0)))
    out = x.astype(np.float32).copy()
    for dh in range(3):
        for dw in range(3):
            out += k[dh, dw] * xp[:, dh : dh + H, dw : dw + W, :]
    return out


def _ensure_ntff_hook():
    """The agent image's antenv lacks axon_hooks; synthesize it so
    run_bass_kernel_spmd(trace=True) can reach the NTFF profiler."""
    import types

    if "antenv.axon_hooks" in sys.modules:
        return
    import antenv

    mod = types.ModuleType("antenv.axon_hooks")
    state = {}
    mod.set_axon_ntff_profile_hook = lambda h: state.__setitem__("h", h)
    mod.get_axon_ntff_profile_hook = lambda: state.get("h")
    sys.modules["antenv.axon_hooks"] = mod
    antenv.axon_hooks = mod
    try:
        if "/root/.axon_site" not in sys.path:
            sys.path.insert(0, "/root/.axon_site")
        from trn_agent_boot.trn_boot import _ntff_profile_via_ctypes

        hook = _ntff_profile_via_ctypes("/opt/axon/libaxon_pjrt.so")
        if hook is not None:
            mod.set_axon_ntff_profile_hook(hook)
    except Exception:
        pass


def _run_on_hw(x, a, beta, trace=False):
    global LAST_RESULTS
    if trace:
        _ensure_ntff_hook()
    import ml_dtypes
    from concourse.bass_utils import run_bass_kernel_spmd

    bf16 = ml_dtypes.bfloat16

    # vertical banded matrix scaled by 0.5: V[i, j] = coeff of x-row i in
    # t'-row j (t' = t/2)
    V = np.zeros((H, H), dtype=np.float32)
    idx = np.arange(H)
    V[idx[:-1] + 1, idx[:-1]] += a[2]
    V[idx, idx] += a[1]
    V[idx[1:] - 1, idx[1:]] += a[0]
    Vb = (0.5 * V).astype(bf16)

    key = (a.tobytes(), float(beta))
    if key not in _CACHE:
        _CACHE[key] = _build_bass(float(beta))
    nc = _CACHE[key]

    # h-major pack: core c gets [112, 4*10752] with images side by side
    xb = np.ascontiguousarray(
        x.reshape(N_CORES, IMGS, H, C).astype(bf16).transpose(0, 2, 1, 3)
    ).reshape(N_CORES, H, COLS)
    in_maps = [{"x": xb[c], "vmat": Vb} for c in range(N_CORES)]
    res = run_bass_kernel_spmd(nc, in_maps, list(range(N_CORES)), trace=trace)
    LAST_RESULTS = res
    # device returned out/2 in bf16 (h-major); unpack + x2 (exact in fp32)
    o = np.stack(
        [np.asarray(res.results[c]["out"]) for c in range(N_CORES)]
    )  # [8, 112, 43008] bf16
    out = (
        o.reshape(N_CORES, H, IMGS, C)
        .transpose(0, 2, 1, 3)
        .astype(np.float32)
        .reshape(N, H, W, CH)
    )
    out *= 2.0
    return out


def kernel(x, kernel=None, _trace=False, **_unused):
    x = np.ascontiguousarray(np.asarray(x, dtype=np.float32))
    assert x.shape == (N, H, W, CH), f"unexpected x shape {x.shape}"
    if kernel is None:
        base = np.array(
            [[1.0, 0.0, -1.0], [0.0, 1.0, 0.0], [-1.0, 0.0, 1.0]], dtype=np.float32
        )
        kernel = np.tile(base[:, :, None, None], (1, 1, 1, CH))
    params = _stencil_params(kernel)
    if params is None:
        return _numpy_fallback(x, kernel)
    a, beta = params
    return _run_on_hw(x, a, beta, trace=_trace)


if __name__ == "__main__":
    xs = np.random.randn(N, H, W, CH).astype(np.float32)
    out = kernel(xs)
    print(out.shape, out.dtype)
